# revision 29
# baseline (speedup 1.0000x reference)
# Dense-MoE (all experts active) Trainium2 kernel.
#
# Two architectures are provided, selectable via MOE_ARCH:
#
#  "dp" (default) — data-parallel: each of the 8 cores owns a 512-token
#   slice and computes ALL 8 experts' 2-layer MLPs over it, streaming the
#   64MB of fp16 expert weights from HBM (119 GB/s sustained, well under
#   the 358 GB/s per-core budget) and accumulating the probs-weighted
#   expert outputs in SBUF. No collectives: no ReduceScatter tail, no
#   cross-core skew stalls, no DMA-queue head-of-line blocking — and the
#   PE sustains its full 2.4GHz clock (collectives were measured to hold
#   it at ~1.95GHz). Startup hides the DVFS ramp behind warmup matmuls,
#   paces the first expert's weight DMAs in consumption order, and runs
#   expert-0's first L1 half k-outer across all 8 PSUM banks so the PE
#   never idles while weights stream in (any idle resets the clock ramp).
#
#  "ep" — expert-parallel (previous baseline): each core computes its
#   expert's dense pass over all tokens, chunked ReduceScatter(add)
#   sums the contributions.
#
# Layout (both): activations transposed on-chip.
#   hT   [IN, tokens]   (host pre-transposes h)
#   hidT [H, tokens] = W1_block.T @ hT per 128-row chunk, gelu+b1 via ACT
#   fe   [tokens, D] = hidT_chunk.T @ W2_chunk accumulated over H chunks
# Matmuls run in float16 (full PE rate, ~4e-4 rel err).
import os
import sys

sys.path.insert(0, "/opt/trn_rl_repo")

import numpy as np

import concourse.mybir as mybir
from concourse import bacc, tile
from concourse import masks
from concourse.bass import _add_dep_helper

B, E, IN, H, D = 4096, 8, 1024, 2048, 1024
NCORES = 8
P = 128
BT = 512                  # tokens per B-tile (ep) / per core (dp)
NBT = B // BT             # 8 B-tiles
NSUB = BT // P            # 4 token sub-tiles per B-tile
KC1 = IN // P             # 8 contraction chunks, layer 1
MC1 = H // P              # 16 H chunks
ND = D // 512             # 2 output column slices of 512

F32 = mybir.dt.float32

_CACHE = {}


# ---------------------------------------------------------------------------
# data-parallel build: per-core inputs are a 512-token hT slice plus ALL
# experts' weights; output is the core's [BT, D] slice of the final result.
# ---------------------------------------------------------------------------
def build_dp(mm_dtype_name="float16", w1_bufs=2, w2_bufs=2):
    mm_dt = getattr(mybir.dt, mm_dtype_name)
    nc = bacc.Bacc("TRN2", target_bir_lowering=False)

    hT = nc.declare_dram_parameter("ht", [IN, BT], mm_dt, isOutput=False)
    w1 = nc.declare_dram_parameter("w1", [E * IN, H], mm_dt, isOutput=False)
    b1t = nc.declare_dram_parameter("b1t", [P, E * MC1], F32, isOutput=False)
    w2 = nc.declare_dram_parameter("w2", [E * H, D], mm_dt, isOutput=False)
    b2s = nc.declare_dram_parameter("b2s", [P, D], F32, isOutput=False)
    out = nc.declare_dram_parameter("out", [BT, D], F32, isOutput=True)

    with tile.TileContext(nc) as tc:
        with (
            tc.tile_pool(name="w1", bufs=w1_bufs) as w1_pool,
            tc.tile_pool(name="w2", bufs=w2_bufs) as w2_pool,
            tc.tile_pool(name="consts", bufs=1) as cpool,
            # hid bufs=1 is stall-free: L1(e+1)'s ACT writes always trail
            # L2(e)'s reads because the PE queue serializes the matmul groups
            tc.tile_pool(name="hid", bufs=1) as hid_pool,
            tc.tile_pool(name="acc", bufs=2) as acc_pool,
            tc.tile_pool(name="l1_ps", bufs=3, space="PSUM") as l1_psum,
            tc.tile_pool(name="l2_ps", bufs=4, space="PSUM") as l2_psum,
            tc.tile_pool(name="ps8", bufs=1, space="PSUM") as ps8_pool,
        ):
            # resident token slice: chunk k at columns [k*BT, (k+1)*BT)
            ht = cpool.tile([P, KC1 * BT], mm_dt, tag="ht")

            # DMA rings fair-share bandwidth across every in-flight transfer
            # with a per-transfer cap of ~55GB/s (aggregate ~350GB/s needs
            # ~7 transfers in flight). An unpaced startup burst (~25MB
            # queued at once) delays slab0 and delivers slabs out of
            # consumption order; a lag-6 completion chain keeps ~6 transfers
            # in flight (near aggregate saturation) while making transfers
            # finish in issue order.
            chain = []
            CHAIN_LAG = 4

            def pdma(dst, src, paced):
                di = nc.sync.dma_start(dst, src)
                if paced:
                    if len(chain) >= CHAIN_LAG:
                        _add_dep_helper(
                            di.ins, chain[-CHAIN_LAG].ins, sync=True,
                            reason="startup dma pacing",
                        )
                    chain.append(di)
                return di

            def dma_w1(e, k, paced=False):
                t_ = w1_pool.tile([P, H], mm_dt, tag=f"w1_{k}")
                pdma(t_[:], w1[e * IN + k * P: e * IN + (k + 1) * P, :], paced)
                return t_

            def dma_w2(e, m, paced=False):
                t_ = w2_pool.tile([P, D], mm_dt, tag=f"w2_{m}")
                pdma(t_[:], w2[e * H + m * P: e * H + (m + 1) * P, :], paced)
                return t_

            # PE warmup: the DVFS governor ramps the PE clock with activity
            # (~5us of matmuls before it reaches 2.4GHz) and resets on any
            # PE idle. Run dummy matmuls on a memset scratch tile during the
            # otherwise-idle startup DMA window so real matmuls start at
            # full clock.
            scratch = cpool.tile([P, 640], mm_dt, tag="warm_src")
            nc.gpsimd.memset(scratch[:], 0.0)
            for w in range(7):
                wps = l1_psum.tile([P, BT], F32, tag="l1")
                nc.tensor.matmul(
                    wps[:], scratch[:, 0:P], scratch[:, P:P + BT],
                    start=True, stop=True,
                )

            # first-needed data first, in consumption order: the k-outer
            # pass A reads only the first column-half of every w1 slab, so
            # (ht_k, w1_k first-half) pairs ship first, then biases, then
            # the second halves for the m-outer pass B
            # pair 0 ships from the idle Scalar queue (hwdge-capable, its
            # queue initializes ~0.6us before Sync's and it has no work
            # until the first gelu at ~27us); with the Sync chain at lag-4
            # the total in-flight stays ~6 transfers (aggregate-saturating)
            w1_sb = []
            for k in range(KC1):
                if k == 0:
                    nc.scalar.dma_start(
                        ht[:, 0:BT], hT[0:P, :]
                    )
                    t_ = w1_pool.tile([P, H], mm_dt, tag=f"w1_{k}")
                    nc.scalar.dma_start(t_[:, 0:H // 2], w1[0:P, 0:H // 2])
                else:
                    pdma(ht[:, k * BT:(k + 1) * BT],
                         hT[k * P:(k + 1) * P, :], True)
                    t_ = w1_pool.tile([P, H], mm_dt, tag=f"w1_{k}")
                    pdma(t_[:, 0:H // 2],
                         w1[k * P:(k + 1) * P, 0:H // 2], True)
                w1_sb.append(t_)
            b1_sb = cpool.tile([P, E * MC1], F32, tag="b1")
            pdma(b1_sb[:], b1t[:], True)
            b2_sb = cpool.tile([P, D], F32, tag="b2")
            pdma(b2_sb[:], b2s[:], True)
            for k in range(KC1):
                pdma(w1_sb[k][:, H // 2:],
                     w1[k * P:(k + 1) * P, H // 2:], True)
            w2_sb = [dma_w2(0, m, paced=True) for m in range(MC1)]

            acc_prev = [None] * NSUB

            for e in range(E):
                # weight prefetch for the NEXT expert rides the pool slack
                # (pool bufs > slabs-per-expert) while this expert computes.
                # e==0's batch joins the paced startup chain (its pool buffers
                # are free at t=0, so it would otherwise compete with the
                # critical first-expert loads).
                if e + 1 < E:
                    paced = e == 0
                    w1_next = [dma_w1(e + 1, k, paced=paced) for k in range(KC1)]
                    w2_next = [dma_w2(e + 1, m, paced=paced) for m in range(MC1)]

                # --- layer 1: hidT chunk m = (W1 block).T @ hT, + b1, gelu ---
                hidA = hid_pool.tile([P, (MC1 // 2) * BT], mm_dt, tag="hidA")
                hidB = hid_pool.tile([P, (MC1 // 2) * BT], mm_dt, tag="hidB")

                def hid_slice(m, lo, hi):
                    half_t = hidA if m < MC1 // 2 else hidB
                    mm_ = m % (MC1 // 2)
                    return half_t[:, mm_ * BT + lo: mm_ * BT + hi]

                def gelu_m(m, ps):
                    nc.scalar.activation(
                        hid_slice(m, 0, BT),
                        ps[:],
                        mybir.ActivationFunctionType.Gelu,
                        bias=b1_sb[:, e * MC1 + m: e * MC1 + m + 1],
                        scale=1.0,
                    )

                if e == 0:
                    # k-outer pass for m=0..6 across 7 PSUM banks: each
                    # k-stage consumes only slab k (+ ht chunk k), matching
                    # the paced startup delivery so the PE never stalls while
                    # the first expert's weights stream in. The 8th bank is
                    # left free so the following m-outer pass can start (m=7)
                    # without waiting for pass A's gelu drain to free a bank.
                    ps7 = (
                        [l1_psum.tile([P, BT], F32, tag="l1", name=f"psA_{i}")
                         for i in range(3)]
                        + [l2_psum.tile([P, BT], F32, tag="l2", name=f"psA_{i + 3}")
                           for i in range(4)]
                    )
                    for k in range(KC1):
                        for mi in range(7):
                            nc.tensor.matmul(
                                ps7[mi][:],
                                w1_sb[k][:, mi * P:(mi + 1) * P],
                                ht[:, k * BT:(k + 1) * BT],
                                start=(k == 0),
                                stop=(k == KC1 - 1),
                            )
                    for mi in range(7):
                        gelu_m(mi, ps7[mi])
                m_lo = 7 if e == 0 else 0
                for m in range(m_lo, MC1):
                    if e == 0 and m == 7:
                        ps = ps8_pool.tile([P, BT], F32, tag="p8")
                    else:
                        ps = l1_psum.tile([P, BT], F32, tag="l1")
                    for k in range(KC1):
                        nc.tensor.matmul(
                            ps[:],
                            w1_sb[k][:, m * P:(m + 1) * P],
                            ht[:, k * BT:(k + 1) * BT],
                            start=(k == 0),
                            stop=(k == KC1 - 1),
                        )
                    gelu_m(m, ps)

                # --- layer 2 + expert accumulation in SBUF ---
                for s in range(NSUB):
                    acc_new = acc_pool.tile([P, D], F32, tag=f"acc{s}")
                    ps_a = l2_psum.tile([P, 512], F32, tag="l2")
                    ps_b = l2_psum.tile([P, 512], F32, tag="l2")
                    pss = [ps_a, ps_b]
                    last = e == E - 1

                    def acc_d(d):
                        cols = slice(d * 512, (d + 1) * 512)
                        other = b2_sb[:, cols] if e == 0 else acc_prev[s][:, cols]
                        nc.vector.tensor_add(acc_new[:, cols], pss[d][:], other)
                        if last:
                            nc.sync.dma_start(
                                out[s * P:(s + 1) * P, cols], acc_new[:, cols]
                            )

                    if last:
                        # sequential d-groups: d0's accumulate + output DMA
                        # overlap d1's matmuls, shortening the kernel tail
                        for d in range(ND):
                            for m in range(MC1):
                                nc.tensor.matmul(
                                    pss[d][:],
                                    hid_slice(m, s * P, (s + 1) * P),
                                    w2_sb[m][:, d * 512:(d + 1) * 512],
                                    start=(m == 0),
                                    stop=(m == MC1 - 1),
                                )
                            acc_d(d)
                    else:
                        for m in range(MC1):
                            hs = hid_slice(m, s * P, (s + 1) * P)
                            for d in range(ND):
                                nc.tensor.matmul(
                                    pss[d][:],
                                    hs,
                                    w2_sb[m][:, d * 512:(d + 1) * 512],
                                    start=(m == 0),
                                    stop=(m == MC1 - 1),
                                )
                        for d in range(ND):
                            acc_d(d)
                    acc_prev[s] = acc_new

                if e + 1 < E:
                    w1_sb = w1_next
                    w2_sb = w2_next

    nc.finalize()
    return nc


# ---------------------------------------------------------------------------
# expert-parallel build (previous baseline, kept as fallback)
# ---------------------------------------------------------------------------
HALF = 2 * P              # 256 rows per ReduceScatter chunk (1 MB)
RS_ROWS = HALF // NCORES  # 32 rows each core receives per RS chunk
NCHUNK = NBT * 2          # 16 RS chunks


def build(mm_dtype_name="float16", nbt=NBT, use_collective=True):
    mm_dt = getattr(mybir.dt, mm_dtype_name)
    bf16 = mybir.dt.size(mm_dt) == 2  # 2-byte path: bf16 or fp16
    nc = bacc.Bacc("TRN2", target_bir_lowering=False)

    if bf16:
        hT = nc.declare_dram_parameter("ht", [IN, nbt * BT], mm_dt, isOutput=False)
    else:
        h = nc.declare_dram_parameter("h", [nbt * BT, IN], F32, isOutput=False)
    w1 = nc.declare_dram_parameter("w1", [IN, H], mm_dt, isOutput=False)
    b1t = nc.declare_dram_parameter("b1t", [P, MC1], F32, isOutput=False)
    w2 = nc.declare_dram_parameter("w2", [H, D], mm_dt, isOutput=False)
    b2b = nc.declare_dram_parameter("b2b", [P, D], F32, isOutput=False)
    out_rows = nbt * BT // NCORES if use_collective else nbt * BT
    out = nc.declare_dram_parameter("out", [out_rows, D], F32, isOutput=True)

    with tile.TileContext(nc) as tc:
        with (
            tc.tile_pool(name="weights", bufs=1) as wpool,
            tc.tile_pool(name="consts", bufs=1) as cpool,
            tc.tile_pool(name="hraw", bufs=2) as hraw_pool,
            tc.tile_pool(name="ht", bufs=(3 if mybir.dt.size(mm_dt) == 2 else 2)) as ht_pool,
            tc.tile_pool(name="hid", bufs=(2 if mybir.dt.size(mm_dt) == 2 else 1)) as hid_pool,
            tc.tile_pool(name="fe", bufs=(2 if mybir.dt.size(mm_dt) == 2 else 1)) as fe_pool,
            tc.tile_pool(name="tp_ps", bufs=(1 if bf16 else 2),
                         space="PSUM") as tp_psum,
            tc.tile_pool(name="l1_ps", bufs=(3 if bf16 else 2),
                         space="PSUM") as l1_psum,
            tc.tile_pool(name="l2_ps", bufs=4, space="PSUM") as l2_psum,
            tc.tile_pool(name="dram", bufs=4, space="DRAM") as dram_pool,
        ):
            hr_pre = []
            ht0 = None
            if bf16:
                ht0 = ht_pool.tile([P, KC1 * BT], mm_dt, tag="ht")
            if not bf16:
                ident = cpool.tile([P, P], F32, tag="ident")
                masks.make_identity(nc, ident[:])

                def prefetch_hr(s):
                    hr = hraw_pool.tile([P, IN], F32, tag="hr")
                    nc.sync.dma_start(hr[:], h[s * P:(s + 1) * P, :])
                    hr_pre.append(hr)

                prefetch_hr(0)
                prefetch_hr(1)

            w1_sb = []
            for k in range(KC1):
                if bf16:
                    nc.sync.dma_start(
                        ht0[:, k * BT:(k + 1) * BT],
                        hT[k * P:(k + 1) * P, 0:BT],
                    )
                t_ = wpool.tile([P, H], mm_dt, tag=f"w1_{k}")
                nc.sync.dma_start(t_[:], w1[k * P:(k + 1) * P, :])
                w1_sb.append(t_)
                if k == 3 and not bf16:
                    prefetch_hr(2)
            if not bf16:
                prefetch_hr(3)
            b1_sb = cpool.tile([P, MC1], F32, tag="b1")
            nc.sync.dma_start(b1_sb[:], b1t[:])
            w2_sb = []
            for m in range(MC1):
                t_ = wpool.tile([P, D], mm_dt, tag=f"w2_{m}")
                nc.sync.dma_start(t_[:], w2[m * P:(m + 1) * P, :])
                w2_sb.append(t_)
            b2_sb = cpool.tile([P, D], F32, tag="b2")
            nc.sync.dma_start(b2_sb[:], b2b[:])

            for t in range(nbt):
                if bf16 and t == 0:
                    ht = ht0
                else:
                    ht = ht_pool.tile([P, KC1 * BT], mm_dt, tag="ht")
                if bf16 and t > 0:
                    for k in range(KC1):
                        nc.sync.dma_start(
                            ht[:, k * BT:(k + 1) * BT],
                            hT[k * P:(k + 1) * P, t * BT:(t + 1) * BT],
                        )
                elif not bf16:
                    for s in range(NSUB):
                        if t == 0:
                            hr = hr_pre[s]
                        else:
                            hr = hraw_pool.tile([P, IN], F32, tag="hr")
                            nc.sync.dma_start(
                                hr[:], h[t * BT + s * P: t * BT + (s + 1) * P, :]
                            )
                        for k in range(KC1):
                            tp = tp_psum.tile([P, P], F32, tag="tp")
                            nc.tensor.transpose(
                                tp[:], hr[:, k * P:(k + 1) * P], ident[:]
                            )
                            nc.vector.tensor_copy(
                                ht[:, k * BT + s * P: k * BT + (s + 1) * P], tp[:]
                            )

                hidA = hid_pool.tile([P, (MC1 // 2) * BT], mm_dt, tag="hidA")
                hidB = hid_pool.tile([P, (MC1 // 2) * BT], mm_dt, tag="hidB")

                def hid_slice(m, lo, hi):
                    half_t = hidA if m < MC1 // 2 else hidB
                    mm_ = m % (MC1 // 2)
                    return half_t[:, mm_ * BT + lo: mm_ * BT + hi]

                for m in range(MC1):
                    ps = l1_psum.tile([P, BT], F32, tag="l1")
                    for k in range(KC1):
                        nc.tensor.matmul(
                            ps[:],
                            w1_sb[k][:, m * P:(m + 1) * P],
                            ht[:, k * BT:(k + 1) * BT],
                            start=(k == 0),
                            stop=(k == KC1 - 1),
                        )
                    nc.scalar.activation(
                        hid_slice(m, 0, BT),
                        ps[:],
                        mybir.ActivationFunctionType.Gelu,
                        bias=b1_sb[:, m:m + 1],
                        scale=1.0,
                    )

                nhalves = 2 if t == nbt - 1 else 1
                subs_per_chunk = NSUB // nhalves
                for half in range(nhalves):
                    fe_chunk = dram_pool.tile(
                        [subs_per_chunk * P, D], F32, tag="fe_dram"
                    )
                    for si in range(subs_per_chunk):
                        s = half * subs_per_chunk + si
                        ps_a = l2_psum.tile([P, 512], F32, tag="l2")
                        ps_b = l2_psum.tile([P, 512], F32, tag="l2")
                        pss = [ps_a, ps_b]
                        for m in range(MC1):
                            hs = hid_slice(m, s * P, (s + 1) * P)
                            for d in range(ND):
                                mi = nc.tensor.matmul(
                                    pss[d][:],
                                    hs,
                                    w2_sb[m][:, d * 512:(d + 1) * 512],
                                    start=(m == 0),
                                    stop=(m == MC1 - 1),
                                )
                                if d > 0:
                                    mi.ins.ldweights = False
                        for d in range(ND):
                            fe_sb = fe_pool.tile([P, 512], F32, tag="fe_sb")
                            nc.vector.tensor_add(
                                fe_sb[:], pss[d][:],
                                b2_sb[:, d * 512:(d + 1) * 512]
                            )
                            nc.sync.dma_start(
                                fe_chunk[si * P:(si + 1) * P,
                                         d * 512:(d + 1) * 512],
                                fe_sb[:],
                            )

                    chunk_rows = subs_per_chunk * P // NCORES
                    row0 = (t * BT + half * subs_per_chunk * P) // NCORES
                    if use_collective:
                        rs_chunk = dram_pool.tile(
                            [chunk_rows, D], F32, tag="rs_dram"
                        )
                        nc.gpsimd.collective_compute(
                            "ReduceScatter",
                            mybir.AluOpType.add,
                            replica_groups=[list(range(NCORES))],
                            ins=[fe_chunk[:]],
                            outs=[rs_chunk[:]],
                        )
                        nc.sync.dma_start(
                            out[row0:row0 + chunk_rows, :], rs_chunk[:]
                        )
                    else:
                        r0 = t * BT + half * subs_per_chunk * P
                        nc.sync.dma_start(
                            out[r0:r0 + subs_per_chunk * P, :], fe_chunk[:]
                        )

    nc.finalize()
    return nc


def _get_nc(arch, mm_dtype_name):
    key = (arch, mm_dtype_name)
    if key not in _CACHE:
        _CACHE[key] = (
            build_dp(mm_dtype_name) if arch == "dp" else build(mm_dtype_name)
        )
    return _CACHE[key]


def _gate_probs(gate_logits):
    z = np.exp(gate_logits - gate_logits.max())
    return (z / z.sum()).astype(np.float32)


def _run_dp(inputs, mm_dtype_name="float16", trace=False):
    from concourse.bass_utils import run_bass_kernel_spmd

    import ml_dtypes

    np_mm = {"bfloat16": ml_dtypes.bfloat16, "float16": np.float16}.get(
        mm_dtype_name, np.float32
    )
    h = np.asarray(inputs["h"], dtype=np.float32)
    hT = np.ascontiguousarray(h.T.astype(np_mm))          # [IN, B]
    gate_logits = np.asarray(inputs["gate_logits"], dtype=np.float64)
    W1 = np.asarray(inputs["W1"], dtype=np.float32)
    b1 = np.asarray(inputs["b1"], dtype=np.float32)
    W2 = np.asarray(inputs["W2"], dtype=np.float32)
    b2 = np.asarray(inputs["b2"], dtype=np.float32)
    probs = _gate_probs(gate_logits)

    w1_all = np.ascontiguousarray(W1.astype(np_mm).reshape(E * IN, H))
    w2_all = np.ascontiguousarray(
        (W2 * probs[:, None, None]).astype(np_mm).reshape(E * H, D)
    )
    b1t_all = np.ascontiguousarray(
        np.concatenate([b1[e].reshape(MC1, P).T for e in range(E)], axis=1)
    )  # [P, E*MC1]
    b2sum = np.ascontiguousarray(
        np.broadcast_to((probs[:, None] * b2).sum(axis=0), (P, D))
    ).astype(np.float32)

    in_maps = []
    for r in range(NCORES):
        ht_r = np.ascontiguousarray(hT[:, r * BT:(r + 1) * BT])
        in_maps.append(
            {"ht": ht_r, "w1": w1_all, "b1t": b1t_all,
             "w2": w2_all, "b2s": b2sum}
        )

    nc = _get_nc("dp", mm_dtype_name)
    res = run_bass_kernel_spmd(nc, in_maps, list(range(NCORES)), trace=trace)

    final = np.empty((B, D), dtype=np.float32)
    for r in range(NCORES):
        final[r * BT:(r + 1) * BT] = res.results[r]["out"]
    return final, res


def _run_ep(inputs, mm_dtype_name="float16", trace=False):
    from concourse.bass_utils import run_bass_kernel_spmd

    import ml_dtypes

    np_mm = {"bfloat16": ml_dtypes.bfloat16, "float16": np.float16}.get(
        mm_dtype_name, np.float32
    )
    bf16 = np_mm != np.float32
    h = np.ascontiguousarray(np.asarray(inputs["h"], dtype=np.float32))
    if bf16:
        h = np.ascontiguousarray(h.T.astype(np_mm))  # pre-transposed [IN, B]
    gate_logits = np.asarray(inputs["gate_logits"], dtype=np.float64)
    W1 = np.asarray(inputs["W1"], dtype=np.float32)
    b1 = np.asarray(inputs["b1"], dtype=np.float32)
    W2 = np.asarray(inputs["W2"], dtype=np.float32)
    b2 = np.asarray(inputs["b2"], dtype=np.float32)
    probs = _gate_probs(gate_logits)

    in_maps = []
    for e in range(NCORES):
        w1_e = np.ascontiguousarray(W1[e].astype(np_mm))         # [IN, H]
        b1t_e = np.ascontiguousarray(b1[e].reshape(MC1, P).T)    # [P, MC1]
        w2_e = np.ascontiguousarray((W2[e] * probs[e]).astype(np_mm))  # [H, D]
        b2b_e = np.ascontiguousarray(
            np.broadcast_to(b2[e] * probs[e], (P, D))
        )
        in_maps.append(
            {("ht" if bf16 else "h"): h, "w1": w1_e, "b1t": b1t_e,
             "w2": w2_e, "b2b": b2b_e}
        )

    nc = _get_nc("ep", mm_dtype_name)
    res = run_bass_kernel_spmd(nc, in_maps, list(range(NCORES)), trace=trace)

    chunks = []          # (global_row0, out_row0, rows_per_core)
    out_pos = 0
    for t in range(NBT):
        nhalves = 2 if t == NBT - 1 else 1
        rows = BT // nhalves
        for half in range(nhalves):
            rpc = rows // NCORES
            chunks.append((t * BT + half * rows, out_pos, rpc))
            out_pos += rpc
    final = np.empty((B, D), dtype=np.float32)
    for r in range(NCORES):
        o = res.results[r]["out"]
        for g0, o0, rpc in chunks:
            final[g0 + r * rpc: g0 + (r + 1) * rpc] = o[o0: o0 + rpc]
    return final, res


def _run(inputs, mm_dtype_name="float16", trace=False, arch=None):
    arch = arch or os.environ.get("MOE_ARCH", "dp")
    if arch == "dp":
        return _run_dp(inputs, mm_dtype_name=mm_dtype_name, trace=trace)
    return _run_ep(inputs, mm_dtype_name=mm_dtype_name, trace=trace)


def kernel(**inputs):
    mm_dtype_name = os.environ.get("MOE_MM_DTYPE", "float16")
    final, _ = _run(inputs, mm_dtype_name=mm_dtype_name, trace=False)
    return final


# revision 31
# speedup vs baseline: 1.0135x; 1.0135x over previous
# Dense-MoE (all experts active) Trainium2 kernel.
#
# Two architectures are provided, selectable via MOE_ARCH:
#
#  "dp" (default) — data-parallel: each of the 8 cores owns a 512-token
#   slice and computes ALL 8 experts' 2-layer MLPs over it, streaming the
#   64MB of fp16 expert weights from HBM (119 GB/s sustained, well under
#   the 358 GB/s per-core budget) and accumulating the probs-weighted
#   expert outputs in SBUF. No collectives: no ReduceScatter tail, no
#   cross-core skew stalls, no DMA-queue head-of-line blocking — and the
#   PE sustains its full 2.4GHz clock (collectives were measured to hold
#   it at ~1.95GHz). Startup hides the DVFS ramp behind warmup matmuls,
#   paces the first expert's weight DMAs in consumption order, and runs
#   expert-0's first L1 half k-outer across all 8 PSUM banks so the PE
#   never idles while weights stream in (any idle resets the clock ramp).
#
#  "ep" — expert-parallel (previous baseline): each core computes its
#   expert's dense pass over all tokens, chunked ReduceScatter(add)
#   sums the contributions.
#
# Layout (both): activations transposed on-chip.
#   hT   [IN, tokens]   (host pre-transposes h)
#   hidT [H, tokens] = W1_block.T @ hT per 128-row chunk, gelu+b1 via ACT
#   fe   [tokens, D] = hidT_chunk.T @ W2_chunk accumulated over H chunks
# Matmuls run in float16 (full PE rate, ~4e-4 rel err).
import os
import sys

sys.path.insert(0, "/opt/trn_rl_repo")

import numpy as np

import concourse.mybir as mybir
from concourse import bacc, tile
from concourse import masks
from concourse.bass import _add_dep_helper

B, E, IN, H, D = 4096, 8, 1024, 2048, 1024
NCORES = 8
P = 128
BT = 512                  # tokens per B-tile (ep) / per core (dp)
NBT = B // BT             # 8 B-tiles
NSUB = BT // P            # 4 token sub-tiles per B-tile
KC1 = IN // P             # 8 contraction chunks, layer 1
MC1 = H // P              # 16 H chunks
ND = D // 512             # 2 output column slices of 512

F32 = mybir.dt.float32

_CACHE = {}


# ---------------------------------------------------------------------------
# data-parallel build: per-core inputs are a 512-token hT slice plus ALL
# experts' weights; output is the core's [BT, D] slice of the final result.
# ---------------------------------------------------------------------------
def build_dp(mm_dtype_name="float16", w1_bufs=2, w2_bufs=2):
    mm_dt = getattr(mybir.dt, mm_dtype_name)
    nc = bacc.Bacc("TRN2", target_bir_lowering=False)

    hT = nc.declare_dram_parameter("ht", [IN, BT], mm_dt, isOutput=False)
    w1 = nc.declare_dram_parameter("w1", [E * IN, H], mm_dt, isOutput=False)
    b1t = nc.declare_dram_parameter("b1t", [P, E * MC1], F32, isOutput=False)
    w2 = nc.declare_dram_parameter("w2", [E * H, D], mm_dt, isOutput=False)
    b2s = nc.declare_dram_parameter("b2s", [P, D], F32, isOutput=False)
    out = nc.declare_dram_parameter("out", [BT, D], F32, isOutput=True)

    with tile.TileContext(nc) as tc:
        with (
            tc.tile_pool(name="w1", bufs=w1_bufs) as w1_pool,
            tc.tile_pool(name="w2", bufs=w2_bufs) as w2_pool,
            tc.tile_pool(name="consts", bufs=1) as cpool,
            # hid bufs=1 is stall-free: L1(e+1)'s ACT writes always trail
            # L2(e)'s reads because the PE queue serializes the matmul groups
            tc.tile_pool(name="hid", bufs=1) as hid_pool,
            tc.tile_pool(name="acc", bufs=2) as acc_pool,
            tc.tile_pool(name="l1_ps", bufs=3, space="PSUM") as l1_psum,
            tc.tile_pool(name="l2_ps", bufs=4, space="PSUM") as l2_psum,
            tc.tile_pool(name="ps8", bufs=1, space="PSUM") as ps8_pool,
        ):
            # resident token slice: chunk k at columns [k*BT, (k+1)*BT)
            ht = cpool.tile([P, KC1 * BT], mm_dt, tag="ht")

            # DMA rings fair-share bandwidth across every in-flight transfer
            # with a per-transfer cap of ~55GB/s (aggregate ~350GB/s needs
            # ~7 transfers in flight). An unpaced startup burst (~25MB
            # queued at once) delays slab0 and delivers slabs out of
            # consumption order; a lag-6 completion chain keeps ~6 transfers
            # in flight (near aggregate saturation) while making transfers
            # finish in issue order.
            chain = []
            CHAIN_LAG = 6

            def pdma(dst, src, paced):
                di = nc.sync.dma_start(dst, src)
                if paced:
                    if len(chain) >= CHAIN_LAG:
                        _add_dep_helper(
                            di.ins, chain[-CHAIN_LAG].ins, sync=True,
                            reason="startup dma pacing",
                        )
                    chain.append(di)
                return di

            def dma_w1(e, k, paced=False):
                t_ = w1_pool.tile([P, H], mm_dt, tag=f"w1_{k}")
                pdma(t_[:], w1[e * IN + k * P: e * IN + (k + 1) * P, :], paced)
                return t_

            def dma_w2(e, m, paced=False):
                t_ = w2_pool.tile([P, D], mm_dt, tag=f"w2_{m}")
                pdma(t_[:], w2[e * H + m * P: e * H + (m + 1) * P, :], paced)
                return t_

            # PE warmup: the DVFS governor ramps the PE clock with activity
            # (~5us of matmuls before it reaches 2.4GHz) and resets on any
            # PE idle. Run dummy matmuls on a memset scratch tile during the
            # otherwise-idle startup DMA window so real matmuls start at
            # full clock.
            scratch = cpool.tile([P, 640], mm_dt, tag="warm_src")
            nc.gpsimd.memset(scratch[:], 0.0)
            for w in range(7):
                wps = l1_psum.tile([P, BT], F32, tag="l1")
                nc.tensor.matmul(
                    wps[:], scratch[:, 0:P], scratch[:, P:P + BT],
                    start=True, stop=True,
                )

            # first-needed data first, in consumption order: the k-outer
            # pass A reads only the first column-half of every w1 slab, so
            # (ht_k, w1_k first-half) pairs ship first, then biases, then
            # the second halves for the m-outer pass B
            w1_sb = []
            for k in range(KC1):
                pdma(ht[:, k * BT:(k + 1) * BT], hT[k * P:(k + 1) * P, :], True)
                t_ = w1_pool.tile([P, H], mm_dt, tag=f"w1_{k}")
                pdma(t_[:, 0:H // 2],
                     w1[k * P:(k + 1) * P, 0:H // 2], True)
                w1_sb.append(t_)
            b1_sb = cpool.tile([P, E * MC1], F32, tag="b1")
            pdma(b1_sb[:], b1t[:], True)
            b2_sb = cpool.tile([P, D], F32, tag="b2")
            pdma(b2_sb[:], b2s[:], True)
            for k in range(KC1):
                pdma(w1_sb[k][:, H // 2:],
                     w1[k * P:(k + 1) * P, H // 2:], True)
            w2_sb = [dma_w2(0, m, paced=True) for m in range(MC1)]

            acc_prev = [None] * NSUB

            for e in range(E):
                # weight prefetch for the NEXT expert rides the pool slack
                # (pool bufs > slabs-per-expert) while this expert computes.
                # e==0's batch joins the paced startup chain (its pool buffers
                # are free at t=0, so it would otherwise compete with the
                # critical first-expert loads).
                if e + 1 < E:
                    paced = e == 0
                    w1_next = [dma_w1(e + 1, k, paced=paced) for k in range(KC1)]
                    w2_next = [dma_w2(e + 1, m, paced=paced) for m in range(MC1)]

                # --- layer 1: hidT chunk m = (W1 block).T @ hT, + b1, gelu ---
                hidA = hid_pool.tile([P, (MC1 // 2) * BT], mm_dt, tag="hidA")
                hidB = hid_pool.tile([P, (MC1 // 2) * BT], mm_dt, tag="hidB")

                def hid_slice(m, lo, hi):
                    half_t = hidA if m < MC1 // 2 else hidB
                    mm_ = m % (MC1 // 2)
                    return half_t[:, mm_ * BT + lo: mm_ * BT + hi]

                def gelu_m(m, ps):
                    nc.scalar.activation(
                        hid_slice(m, 0, BT),
                        ps[:],
                        mybir.ActivationFunctionType.Gelu,
                        bias=b1_sb[:, e * MC1 + m: e * MC1 + m + 1],
                        scale=1.0,
                    )

                if e == 0:
                    # k-outer pass for m=0..6 across 7 PSUM banks: each
                    # k-stage consumes only slab k (+ ht chunk k), matching
                    # the paced startup delivery so the PE never stalls while
                    # the first expert's weights stream in. The 8th bank is
                    # left free so the following m-outer pass can start (m=7)
                    # without waiting for pass A's gelu drain to free a bank.
                    ps7 = (
                        [l1_psum.tile([P, BT], F32, tag="l1", name=f"psA_{i}")
                         for i in range(3)]
                        + [l2_psum.tile([P, BT], F32, tag="l2", name=f"psA_{i + 3}")
                           for i in range(4)]
                    )
                    for k in range(KC1):
                        for mi in range(7):
                            nc.tensor.matmul(
                                ps7[mi][:],
                                w1_sb[k][:, mi * P:(mi + 1) * P],
                                ht[:, k * BT:(k + 1) * BT],
                                start=(k == 0),
                                stop=(k == KC1 - 1),
                            )
                    for mi in range(7):
                        gelu_m(mi, ps7[mi])
                m_lo = 7 if e == 0 else 0
                for m in range(m_lo, MC1):
                    if e == 0 and m == 7:
                        ps = ps8_pool.tile([P, BT], F32, tag="p8")
                    else:
                        ps = l1_psum.tile([P, BT], F32, tag="l1")
                    for k in range(KC1):
                        nc.tensor.matmul(
                            ps[:],
                            w1_sb[k][:, m * P:(m + 1) * P],
                            ht[:, k * BT:(k + 1) * BT],
                            start=(k == 0),
                            stop=(k == KC1 - 1),
                        )
                    gelu_m(m, ps)

                # --- layer 2 + expert accumulation in SBUF ---
                for s in range(NSUB):
                    acc_new = acc_pool.tile([P, D], F32, tag=f"acc{s}")
                    ps_a = l2_psum.tile([P, 512], F32, tag="l2")
                    ps_b = l2_psum.tile([P, 512], F32, tag="l2")
                    pss = [ps_a, ps_b]
                    last = e == E - 1

                    def acc_d(d):
                        cols = slice(d * 512, (d + 1) * 512)
                        other = b2_sb[:, cols] if e == 0 else acc_prev[s][:, cols]
                        nc.vector.tensor_add(acc_new[:, cols], pss[d][:], other)
                        if last:
                            nc.sync.dma_start(
                                out[s * P:(s + 1) * P, cols], acc_new[:, cols]
                            )

                    if last:
                        # sequential d-groups: d0's accumulate + output DMA
                        # overlap d1's matmuls, shortening the kernel tail
                        for d in range(ND):
                            for m in range(MC1):
                                nc.tensor.matmul(
                                    pss[d][:],
                                    hid_slice(m, s * P, (s + 1) * P),
                                    w2_sb[m][:, d * 512:(d + 1) * 512],
                                    start=(m == 0),
                                    stop=(m == MC1 - 1),
                                )
                            acc_d(d)
                    else:
                        for m in range(MC1):
                            hs = hid_slice(m, s * P, (s + 1) * P)
                            for d in range(ND):
                                nc.tensor.matmul(
                                    pss[d][:],
                                    hs,
                                    w2_sb[m][:, d * 512:(d + 1) * 512],
                                    start=(m == 0),
                                    stop=(m == MC1 - 1),
                                )
                        for d in range(ND):
                            acc_d(d)
                    acc_prev[s] = acc_new

                if e + 1 < E:
                    w1_sb = w1_next
                    w2_sb = w2_next

    nc.finalize()
    return nc


# ---------------------------------------------------------------------------
# expert-parallel build (previous baseline, kept as fallback)
# ---------------------------------------------------------------------------
HALF = 2 * P              # 256 rows per ReduceScatter chunk (1 MB)
RS_ROWS = HALF // NCORES  # 32 rows each core receives per RS chunk
NCHUNK = NBT * 2          # 16 RS chunks


def build(mm_dtype_name="float16", nbt=NBT, use_collective=True):
    mm_dt = getattr(mybir.dt, mm_dtype_name)
    bf16 = mybir.dt.size(mm_dt) == 2  # 2-byte path: bf16 or fp16
    nc = bacc.Bacc("TRN2", target_bir_lowering=False)

    if bf16:
        hT = nc.declare_dram_parameter("ht", [IN, nbt * BT], mm_dt, isOutput=False)
    else:
        h = nc.declare_dram_parameter("h", [nbt * BT, IN], F32, isOutput=False)
    w1 = nc.declare_dram_parameter("w1", [IN, H], mm_dt, isOutput=False)
    b1t = nc.declare_dram_parameter("b1t", [P, MC1], F32, isOutput=False)
    w2 = nc.declare_dram_parameter("w2", [H, D], mm_dt, isOutput=False)
    b2b = nc.declare_dram_parameter("b2b", [P, D], F32, isOutput=False)
    out_rows = nbt * BT // NCORES if use_collective else nbt * BT
    out = nc.declare_dram_parameter("out", [out_rows, D], F32, isOutput=True)

    with tile.TileContext(nc) as tc:
        with (
            tc.tile_pool(name="weights", bufs=1) as wpool,
            tc.tile_pool(name="consts", bufs=1) as cpool,
            tc.tile_pool(name="hraw", bufs=2) as hraw_pool,
            tc.tile_pool(name="ht", bufs=(3 if mybir.dt.size(mm_dt) == 2 else 2)) as ht_pool,
            tc.tile_pool(name="hid", bufs=(2 if mybir.dt.size(mm_dt) == 2 else 1)) as hid_pool,
            tc.tile_pool(name="fe", bufs=(2 if mybir.dt.size(mm_dt) == 2 else 1)) as fe_pool,
            tc.tile_pool(name="tp_ps", bufs=(1 if bf16 else 2),
                         space="PSUM") as tp_psum,
            tc.tile_pool(name="l1_ps", bufs=(3 if bf16 else 2),
                         space="PSUM") as l1_psum,
            tc.tile_pool(name="l2_ps", bufs=4, space="PSUM") as l2_psum,
            tc.tile_pool(name="dram", bufs=4, space="DRAM") as dram_pool,
        ):
            hr_pre = []
            ht0 = None
            if bf16:
                ht0 = ht_pool.tile([P, KC1 * BT], mm_dt, tag="ht")
            if not bf16:
                ident = cpool.tile([P, P], F32, tag="ident")
                masks.make_identity(nc, ident[:])

                def prefetch_hr(s):
                    hr = hraw_pool.tile([P, IN], F32, tag="hr")
                    nc.sync.dma_start(hr[:], h[s * P:(s + 1) * P, :])
                    hr_pre.append(hr)

                prefetch_hr(0)
                prefetch_hr(1)

            w1_sb = []
            for k in range(KC1):
                if bf16:
                    nc.sync.dma_start(
                        ht0[:, k * BT:(k + 1) * BT],
                        hT[k * P:(k + 1) * P, 0:BT],
                    )
                t_ = wpool.tile([P, H], mm_dt, tag=f"w1_{k}")
                nc.sync.dma_start(t_[:], w1[k * P:(k + 1) * P, :])
                w1_sb.append(t_)
                if k == 3 and not bf16:
                    prefetch_hr(2)
            if not bf16:
                prefetch_hr(3)
            b1_sb = cpool.tile([P, MC1], F32, tag="b1")
            nc.sync.dma_start(b1_sb[:], b1t[:])
            w2_sb = []
            for m in range(MC1):
                t_ = wpool.tile([P, D], mm_dt, tag=f"w2_{m}")
                nc.sync.dma_start(t_[:], w2[m * P:(m + 1) * P, :])
                w2_sb.append(t_)
            b2_sb = cpool.tile([P, D], F32, tag="b2")
            nc.sync.dma_start(b2_sb[:], b2b[:])

            for t in range(nbt):
                if bf16 and t == 0:
                    ht = ht0
                else:
                    ht = ht_pool.tile([P, KC1 * BT], mm_dt, tag="ht")
                if bf16 and t > 0:
                    for k in range(KC1):
                        nc.sync.dma_start(
                            ht[:, k * BT:(k + 1) * BT],
                            hT[k * P:(k + 1) * P, t * BT:(t + 1) * BT],
                        )
                elif not bf16:
                    for s in range(NSUB):
                        if t == 0:
                            hr = hr_pre[s]
                        else:
                            hr = hraw_pool.tile([P, IN], F32, tag="hr")
                            nc.sync.dma_start(
                                hr[:], h[t * BT + s * P: t * BT + (s + 1) * P, :]
                            )
                        for k in range(KC1):
                            tp = tp_psum.tile([P, P], F32, tag="tp")
                            nc.tensor.transpose(
                                tp[:], hr[:, k * P:(k + 1) * P], ident[:]
                            )
                            nc.vector.tensor_copy(
                                ht[:, k * BT + s * P: k * BT + (s + 1) * P], tp[:]
                            )

                hidA = hid_pool.tile([P, (MC1 // 2) * BT], mm_dt, tag="hidA")
                hidB = hid_pool.tile([P, (MC1 // 2) * BT], mm_dt, tag="hidB")

                def hid_slice(m, lo, hi):
                    half_t = hidA if m < MC1 // 2 else hidB
                    mm_ = m % (MC1 // 2)
                    return half_t[:, mm_ * BT + lo: mm_ * BT + hi]

                for m in range(MC1):
                    ps = l1_psum.tile([P, BT], F32, tag="l1")
                    for k in range(KC1):
                        nc.tensor.matmul(
                            ps[:],
                            w1_sb[k][:, m * P:(m + 1) * P],
                            ht[:, k * BT:(k + 1) * BT],
                            start=(k == 0),
                            stop=(k == KC1 - 1),
                        )
                    nc.scalar.activation(
                        hid_slice(m, 0, BT),
                        ps[:],
                        mybir.ActivationFunctionType.Gelu,
                        bias=b1_sb[:, m:m + 1],
                        scale=1.0,
                    )

                nhalves = 2 if t == nbt - 1 else 1
                subs_per_chunk = NSUB // nhalves
                for half in range(nhalves):
                    fe_chunk = dram_pool.tile(
                        [subs_per_chunk * P, D], F32, tag="fe_dram"
                    )
                    for si in range(subs_per_chunk):
                        s = half * subs_per_chunk + si
                        ps_a = l2_psum.tile([P, 512], F32, tag="l2")
                        ps_b = l2_psum.tile([P, 512], F32, tag="l2")
                        pss = [ps_a, ps_b]
                        for m in range(MC1):
                            hs = hid_slice(m, s * P, (s + 1) * P)
                            for d in range(ND):
                                mi = nc.tensor.matmul(
                                    pss[d][:],
                                    hs,
                                    w2_sb[m][:, d * 512:(d + 1) * 512],
                                    start=(m == 0),
                                    stop=(m == MC1 - 1),
                                )
                                if d > 0:
                                    mi.ins.ldweights = False
                        for d in range(ND):
                            fe_sb = fe_pool.tile([P, 512], F32, tag="fe_sb")
                            nc.vector.tensor_add(
                                fe_sb[:], pss[d][:],
                                b2_sb[:, d * 512:(d + 1) * 512]
                            )
                            nc.sync.dma_start(
                                fe_chunk[si * P:(si + 1) * P,
                                         d * 512:(d + 1) * 512],
                                fe_sb[:],
                            )

                    chunk_rows = subs_per_chunk * P // NCORES
                    row0 = (t * BT + half * subs_per_chunk * P) // NCORES
                    if use_collective:
                        rs_chunk = dram_pool.tile(
                            [chunk_rows, D], F32, tag="rs_dram"
                        )
                        nc.gpsimd.collective_compute(
                            "ReduceScatter",
                            mybir.AluOpType.add,
                            replica_groups=[list(range(NCORES))],
                            ins=[fe_chunk[:]],
                            outs=[rs_chunk[:]],
                        )
                        nc.sync.dma_start(
                            out[row0:row0 + chunk_rows, :], rs_chunk[:]
                        )
                    else:
                        r0 = t * BT + half * subs_per_chunk * P
                        nc.sync.dma_start(
                            out[r0:r0 + subs_per_chunk * P, :], fe_chunk[:]
                        )

    nc.finalize()
    return nc


def _get_nc(arch, mm_dtype_name):
    key = (arch, mm_dtype_name)
    if key not in _CACHE:
        _CACHE[key] = (
            build_dp(mm_dtype_name) if arch == "dp" else build(mm_dtype_name)
        )
    return _CACHE[key]


def _gate_probs(gate_logits):
    z = np.exp(gate_logits - gate_logits.max())
    return (z / z.sum()).astype(np.float32)


def _run_dp(inputs, mm_dtype_name="float16", trace=False):
    from concourse.bass_utils import run_bass_kernel_spmd

    import ml_dtypes

    np_mm = {"bfloat16": ml_dtypes.bfloat16, "float16": np.float16}.get(
        mm_dtype_name, np.float32
    )
    h = np.asarray(inputs["h"], dtype=np.float32)
    hT = np.ascontiguousarray(h.T.astype(np_mm))          # [IN, B]
    gate_logits = np.asarray(inputs["gate_logits"], dtype=np.float64)
    W1 = np.asarray(inputs["W1"], dtype=np.float32)
    b1 = np.asarray(inputs["b1"], dtype=np.float32)
    W2 = np.asarray(inputs["W2"], dtype=np.float32)
    b2 = np.asarray(inputs["b2"], dtype=np.float32)
    probs = _gate_probs(gate_logits)

    w1_all = np.ascontiguousarray(W1.astype(np_mm).reshape(E * IN, H))
    w2_all = np.ascontiguousarray(
        (W2 * probs[:, None, None]).astype(np_mm).reshape(E * H, D)
    )
    b1t_all = np.ascontiguousarray(
        np.concatenate([b1[e].reshape(MC1, P).T for e in range(E)], axis=1)
    )  # [P, E*MC1]
    b2sum = np.ascontiguousarray(
        np.broadcast_to((probs[:, None] * b2).sum(axis=0), (P, D))
    ).astype(np.float32)

    in_maps = []
    for r in range(NCORES):
        ht_r = np.ascontiguousarray(hT[:, r * BT:(r + 1) * BT])
        in_maps.append(
            {"ht": ht_r, "w1": w1_all, "b1t": b1t_all,
             "w2": w2_all, "b2s": b2sum}
        )

    nc = _get_nc("dp", mm_dtype_name)
    res = run_bass_kernel_spmd(nc, in_maps, list(range(NCORES)), trace=trace)

    final = np.empty((B, D), dtype=np.float32)
    for r in range(NCORES):
        final[r * BT:(r + 1) * BT] = res.results[r]["out"]
    return final, res


def _run_ep(inputs, mm_dtype_name="float16", trace=False):
    from concourse.bass_utils import run_bass_kernel_spmd

    import ml_dtypes

    np_mm = {"bfloat16": ml_dtypes.bfloat16, "float16": np.float16}.get(
        mm_dtype_name, np.float32
    )
    bf16 = np_mm != np.float32
    h = np.ascontiguousarray(np.asarray(inputs["h"], dtype=np.float32))
    if bf16:
        h = np.ascontiguousarray(h.T.astype(np_mm))  # pre-transposed [IN, B]
    gate_logits = np.asarray(inputs["gate_logits"], dtype=np.float64)
    W1 = np.asarray(inputs["W1"], dtype=np.float32)
    b1 = np.asarray(inputs["b1"], dtype=np.float32)
    W2 = np.asarray(inputs["W2"], dtype=np.float32)
    b2 = np.asarray(inputs["b2"], dtype=np.float32)
    probs = _gate_probs(gate_logits)

    in_maps = []
    for e in range(NCORES):
        w1_e = np.ascontiguousarray(W1[e].astype(np_mm))         # [IN, H]
        b1t_e = np.ascontiguousarray(b1[e].reshape(MC1, P).T)    # [P, MC1]
        w2_e = np.ascontiguousarray((W2[e] * probs[e]).astype(np_mm))  # [H, D]
        b2b_e = np.ascontiguousarray(
            np.broadcast_to(b2[e] * probs[e], (P, D))
        )
        in_maps.append(
            {("ht" if bf16 else "h"): h, "w1": w1_e, "b1t": b1t_e,
             "w2": w2_e, "b2b": b2b_e}
        )

    nc = _get_nc("ep", mm_dtype_name)
    res = run_bass_kernel_spmd(nc, in_maps, list(range(NCORES)), trace=trace)

    chunks = []          # (global_row0, out_row0, rows_per_core)
    out_pos = 0
    for t in range(NBT):
        nhalves = 2 if t == NBT - 1 else 1
        rows = BT // nhalves
        for half in range(nhalves):
            rpc = rows // NCORES
            chunks.append((t * BT + half * rows, out_pos, rpc))
            out_pos += rpc
    final = np.empty((B, D), dtype=np.float32)
    for r in range(NCORES):
        o = res.results[r]["out"]
        for g0, o0, rpc in chunks:
            final[g0 + r * rpc: g0 + (r + 1) * rpc] = o[o0: o0 + rpc]
    return final, res


def _run(inputs, mm_dtype_name="float16", trace=False, arch=None):
    arch = arch or os.environ.get("MOE_ARCH", "dp")
    if arch == "dp":
        return _run_dp(inputs, mm_dtype_name=mm_dtype_name, trace=trace)
    return _run_ep(inputs, mm_dtype_name=mm_dtype_name, trace=trace)


def kernel(**inputs):
    mm_dtype_name = os.environ.get("MOE_MM_DTYPE", "float16")
    final, _ = _run(inputs, mm_dtype_name=mm_dtype_name, trace=False)
    return final
